# revision 20
# baseline (speedup 1.0000x reference)
"""Kalman filter (state=16, obs=96, T=8192) on 8 Trainium2 NeuronCores.

Math: with isotropic A=alpha*I, Q=q*I, R=r*I, P0=p0*I the whole Riccati
trajectory is diagonal in the fixed orthonormal eigenbasis U of C^T C
(SVD C = Z diag(sig) U^T).  The filter reduces to 16 independent scalar
recurrences z_t = a_t * z_{t-1} + g_t * (Z^T y_t), x_t = U z_t, with
a_t, g_t from a scalar per-mode Riccati recursion (y-independent, host
precomputed in fp64).

Device work is minimized via the substitution zeta_t = z_t / g_t:
    zeta_t = a'_t * zeta_{t-1} + w_t,   a'_t = a_t * g_{t-1} / g_t,
    w = Z^T y.
a' converges geometrically to a steady state a_ss; the device runs the
whole scan with a_ss (broadcast from 2 bitcast bf16 columns riding in
the input DMA) and the host recomputes the transient prefix (t < TRH)
exactly in fp64.

Per core the device does: two input DMAs (bf16 [Z | a_ss | y], split
across the SP HWDGE and Pool SWDGE queues), nine bf16 matmuls into
PSUM, and three independent zero-init fp32 prefix scans
(tensor_tensor_scan) on DVE.  The scans are partition-FOLDED: matmul
output base partitions may be {0,32,64}, so three consecutive time
blocks land at psum partitions {0:16,32:48,64:80} of one tile and one
scan instruction advances all three in parallel (1/3 the serial
length).  One merged bf16 output DMA (rows >= 512B so the DMA engines
run at full rate) returns all pieces.  All pieces start from zero; the
host stitches the carries of the 9 virtual chunks per core, applies
the g-multiply and the U@z rotation in a tiny [T,16] fp64 post-pass,
and discards the unused partition rows.  Synchronization is
hand-rolled semaphores (no TileContext).
"""

import numpy as np

STATE = 16
OBS = 96
T = 8192
N_CORES = 8
L = T // N_CORES   # 1024 steps per core
SB = 83            # piece 0 fold block width  (cols 0:249, 3 x 83)
FB = 88            # piece 1 fold block width  (cols 249:513, 3 x 88)
XB = 171           # piece 2 fold block width  (cols 513:1026, 2 junk cols)
ZC = 344           # zout columns: SB + FB + XB = 342, padded to 344
TRH = 513          # host-exact transient prefix (a' not converged before)

_COMPILED = {}


def _build_nc():
    from concourse import bacc, mybir

    f32 = mybir.dt.float32
    bf16 = mybir.dt.bfloat16
    mult, add = mybir.AluOpType.mult, mybir.AluOpType.add
    nc = bacc.Bacc("TRN2", target_bir_lowering=False, debug=False,
                   num_devices=N_CORES)
    # yz layout: [Z(0:16) | a_ss bitcast(16:18) | y(18:1042) | junk(1042:1046)]
    yz_d = nc.dram_tensor("yz", [OBS, 18 + L + 4], bf16, kind="ExternalInput")
    z_d = nc.dram_tensor("zT", [80, ZC], bf16, kind="ExternalOutput")

    s_a = nc.alloc_semaphore("s_a")      # chunkA DMA completion
    s_b = nc.alloc_semaphore("s_b")      # chunkB DMA completion
    s_mm = nc.alloc_semaphore("s_mm")    # matmul progress
    s_sc = nc.alloc_semaphore("s_sc")    # scan progress
    s_out = nc.alloc_semaphore("s_out")  # output DMA completions

    yzA = nc.alloc_sbuf_tensor("yzA", [OBS, 532], bf16)
    yzB = nc.alloc_sbuf_tensor("yzB", [OBS, 514], bf16)
    zout = nc.alloc_sbuf_tensor("zout", [80, ZC], bf16)
    wp0 = nc.alloc_psum_tensor("wp0", [80, SB], f32)
    wpP = nc.alloc_psum_tensor("wpP", [80, FB], f32)
    wpX = nc.alloc_psum_tensor("wpX", [80, XB], f32)

    nc.sync.dma_start(yzA[:, :], yz_d[:, 0:532]).then_inc(s_a, 16)
    nc.gpsimd.dma_start(yzB[:, :], yz_d[:, 530:1044]).then_inc(s_b, 16)

    zt = yzA[:, 0:16]
    nc.tensor.wait_ge(s_a, 16)
    for b in range(3):
        lo = 18 + b * SB
        nc.tensor.matmul(wp0[32 * b:32 * b + 16, :], zt, yzA[:, lo:lo + SB],
                         start=True, stop=True).then_inc(s_mm, 1)
    for b in range(3):
        lo = 18 + 3 * SB + b * FB
        nc.tensor.matmul(wpP[32 * b:32 * b + 16, :], zt, yzA[:, lo:lo + FB],
                         start=True, stop=True).then_inc(s_mm, 1)
    nc.tensor.wait_ge(s_b, 16)
    for b in range(3):
        nc.tensor.matmul(wpX[32 * b:32 * b + 16, :], zt,
                         yzB[:, 1 + b * XB:1 + (b + 1) * XB],
                         start=True, stop=True).then_inc(s_mm, 1)

    # psum rows 16:32 / 48:64 are never written; the scans compute garbage
    # there and the host drops those rows — harmless on hardware.
    def a_bc(p, n):
        return yzA[0:p, 16:18].bitcast(f32).broadcast_to([p, n])

    nc.vector.wait_ge(s_mm, 3)
    nc.vector.tensor_tensor_scan(zout[:, 0:SB], a_bc(80, SB), wp0[:, :],
                                 0.0, mult, add).then_inc(s_sc, 1)
    nc.vector.wait_ge(s_mm, 6)
    nc.vector.tensor_tensor_scan(zout[:, SB:SB + FB], a_bc(80, FB),
                                 wpP[:, :], 0.0, mult, add).then_inc(s_sc, 1)
    nc.vector.wait_ge(s_mm, 9)
    nc.vector.tensor_tensor_scan(zout[:, SB + FB:SB + FB + XB], a_bc(80, XB),
                                 wpX[:, :], 0.0, mult, add).then_inc(s_sc, 1)

    nc.sync.wait_ge(s_sc, 3)
    nc.sync.dma_start(z_d[:, :], zout[:, :]).then_inc(s_out, 16)
    nc.sync.wait_ge(s_out, 16)

    nc.compile()
    return nc


# per-core virtual scan pieces (lo, hi) in local time
_PIECES = [(b * SB, (b + 1) * SB) for b in range(3)] + \
          [(3 * SB + b * FB, 3 * SB + (b + 1) * FB) for b in range(3)] + \
          [(513 + b * XB, min(513 + (b + 1) * XB, L)) for b in range(3)]


def _host_precompute(A, C, Q, R, x_init, P_init):
    """fp64 y-independent precompute: SVD of C + per-mode scalar Riccati."""
    A64 = A.astype(np.float64)
    C64 = C.astype(np.float64)
    alpha = A64[0, 0]
    q = Q.astype(np.float64)[0, 0]
    r = R.astype(np.float64)[0, 0]
    p0 = P_init.astype(np.float64)[0, 0]

    Zs, sig, UT = np.linalg.svd(C64, full_matrices=False)
    U = UT.T

    d = np.full(STATE, p0)
    a_seq = np.empty((T, STATE))
    g_seq = np.empty((T, STATE))
    for t in range(T):
        dp = alpha * alpha * d + q
        g = dp * sig / (sig * sig * dp + r)
        oneminus = 1.0 - sig * g
        a_seq[t] = alpha * oneminus
        g_seq[t] = g
        d = oneminus * dp

    # zeta-space decay a'_t = a_t * g_{t-1} / g_t (g_{-1} := g_0) and its
    # steady state (device uses fp32 a_ss everywhere)
    g_prev = np.vstack([g_seq[:1], g_seq[:-1]])
    ap_seq = a_seq * g_prev / g_seq
    a_ss = ap_seq[-1].astype(np.float32).astype(np.float64)

    # prefix products of a_ss, long enough for the longest piece
    pi_ss = np.cumprod(np.broadcast_to(a_ss, (512, STATE)), axis=0)

    z0 = U.T @ x_init.astype(np.float64)
    return Zs, U, a_seq, g_seq, pi_ss, z0


def _isotropic(M, dim):
    c = M[0, 0]
    return bool(np.abs(M - c * np.eye(dim, dtype=M.dtype)).max() <= 1e-30)


def _fallback(y_seq, A, C, Q, R, x_init, P_init):
    """General (non-isotropic) inputs: plain fp32 numpy filter."""
    f = np.float32
    A = A.astype(f); C = C.astype(f); Q = Q.astype(f); R = R.astype(f)
    x = x_init.astype(f); P = P_init.astype(f)
    I = np.eye(STATE, dtype=f)
    out = np.empty((T, STATE), f)
    for t in range(T):
        x_pred = A @ x
        P_pred = A @ P @ A.T + Q
        S = C @ P_pred @ C.T + R
        K = (P_pred @ C.T @ np.linalg.inv(S)).astype(f)
        x = x_pred + K @ (y_seq[t].astype(f) - C @ x_pred)
        P = ((I - K @ C) @ P_pred).astype(f)
        out[t] = x
    return out


def _to_bf16(x):
    x = np.ascontiguousarray(x, np.float32)
    u = x.view(np.uint32)
    return ((u + 0x7FFF + ((u >> 16) & 1)) & 0xFFFF0000).view(np.float32)


def kernel(y_seq, A, C, Q, R, x_init, P_init):
    y_seq = np.asarray(y_seq)
    A = np.asarray(A); C = np.asarray(C); Q = np.asarray(Q)
    R = np.asarray(R)
    x_init = np.asarray(x_init); P_init = np.asarray(P_init)

    if not (_isotropic(A, STATE) and _isotropic(Q, STATE)
            and _isotropic(R, OBS) and _isotropic(P_init, STATE)):
        return _fallback(y_seq, A, C, Q, R, x_init, P_init)

    Zs, U, a_seq, g_seq, pi_ss, z0 = _host_precompute(
        A, C, Q, R, x_init, P_init)

    if "nc" not in _COMPILED:
        _COMPILED["nc"] = _build_nc()
    nc = _COMPILED["nc"]

    import ml_dtypes
    f = np.float32
    Zb = np.ascontiguousarray(Zs, f)
    a_ss32 = pi_ss[0].astype(f)
    # a_ss replicated down all partitions: row r holds a_ss[r mod 16] so the
    # folded scans read their per-mode decay at partitions 32b+m
    a_rep = np.tile(a_ss32, OBS // STATE)[:, None]

    in_maps = []
    for c in range(N_CORES):
        sl = slice(c * L, (c + 1) * L)
        yz = np.zeros((OBS, 18 + L + 4), f)
        yz[:, :16] = Zb
        yz[:, 18:18 + L] = y_seq[sl].T
        yz16 = _to_bf16(yz).astype(ml_dtypes.bfloat16)
        yz16[:, 16:18] = a_rep.view(ml_dtypes.bfloat16)
        in_maps.append({"yz": yz16})

    from concourse.bass_utils import run_bass_kernel_spmd
    res = run_bass_kernel_spmd(nc, in_maps, core_ids=list(range(N_CORES)))

    # unscramble the folded device layout into [T,16] zeta (fp64)
    zeta = np.empty((T, STATE))
    for c in range(N_CORES):
        zT = res.results[c]["zT"].astype(np.float64)  # [80, ZC]
        base = c * L
        for b in range(3):
            zeta[base + b * SB:base + (b + 1) * SB] = \
                zT[32 * b:32 * b + 16, 0:SB].T
        for b in range(3):
            lo = base + 3 * SB + b * FB
            zeta[lo:lo + FB] = zT[32 * b:32 * b + 16, SB:SB + FB].T
        for b in range(3):
            lo, hi = 513 + b * XB, min(513 + (b + 1) * XB, L)
            zeta[base + lo:base + hi] = zT[32 * b:32 * b + 16,
                                           SB + FB:SB + FB + hi - lo].T

    # host post-pass (fp64): exact transient prefix, carry stitch across the
    # unchained pieces, then z = g * zeta and x = z @ U^T
    w0 = y_seq[:TRH].astype(np.float64) @ Zs
    zp = z0
    zexact = np.empty((TRH, STATE))
    for t in range(TRH):
        zp = a_seq[t] * zp + g_seq[t] * w0[t]
        zexact[t] = zp
    zeta[:TRH] = zexact / g_seq[:TRH]

    carry = zeta[TRH - 1]
    for c in range(N_CORES):
        for lo, hi in _PIECES:
            if c == 0 and hi <= TRH:
                continue  # host-exact prefix already has its carry folded in
            sl = slice(c * L + lo, c * L + hi)
            zeta[sl] += pi_ss[:hi - lo] * carry[None, :]
            carry = zeta[c * L + hi - 1]

    x = (g_seq * zeta) @ U.T
    return x.astype(f)
